# revision 1
# baseline (speedup 1.0000x reference)
"""Causal single-head attention (B=4, S=4096, D=1024, d_key=64) on 8 trn2 cores.

Sharding: 8 cores = 4 batches x 2 query-interleave halves. Core (b, h) handles
batch b and query chunks of 256 rows at global chunk indices {2g+h : g=0..7}
(interleaved for causal load balance). Keys/values for the batch are replicated
on both cores of the pair.

Device kernel (identical SPMD program; per-core differences are input data):
  1. Project kT [64, S] and qT [64, 2048] (weights as lhsT) and v-natural
     [128, 65] blocks (data chunk as lhsT, weights as rhs -- emits the PV
     layout directly, no transposes; a DMA'd ones column provides softmax
     denominators) from host-pre-transposed bf16 KT/VT/QT [1024, S] and
     W*T [1024, 64] (bf16 matmuls, fp32 accumulate). kT/qT are stored fp32r
     so attention matmuls run at full precision-speed (1 cycle/row, N>=256).
  2. KEY-MAJOR attention: for each key group t (512 keys = one "quad" of 4
     key blocks), right after k/v group t is projected, every query chunk
     g >= t computes its transposed scores sT[j, i] = k_j . q_i over that
     group (one [128, 1024] PSUM tile / one ACT exp with scale=1/8 per quad),
     applies the causal boundary mask (host-built multiplicative [128, 1024] tile) when t == g,
     then PV-accumulates the quad into PSUM [65, 256] and DVE-adds it into a
     per-chunk SBUF accumulator osb[:, g, :] whose row 64 is the softmax
     denominator. Only quad (7,7) depends on the final input DMA, so the
     post-DMA tail is tiny.
  3. Per chunk, right after its boundary group: DMA the raw accumulator to
     DRAM via the POOL DGE; the host divides by the denominator row and
     transposes (a few MB of numpy).
"""

import numpy as np

import concourse.mybir as mybir
import concourse.tile as tile
from concourse import bacc
from concourse.bass_utils import run_bass_kernel_spmd

B, S, D, DK = 4, 4096, 1024, 64
NCORES = 8
CH = 256  # query rows per chunk
NCH = 8  # chunks per core
QROWS = CH * NCH  # 2048 query rows per core
JB = 128  # key block
DC = D // 128  # 8 contraction chunks
F32 = mybir.dt.float32
F32R = mybir.dt.float32r
BF16 = mybir.dt.bfloat16

_prog_cache = {}
_last_in_maps = None


def _build(variant):
    causal = variant == "causal"
    # number of key quads (4 key blocks of 128 = 512 keys) per chunk
    nq = [g + 1 for g in range(NCH)] if causal else [S // 512] * NCH

    nc = bacc.Bacc("TRN2", target_bir_lowering=False, debug=False,
                   num_devices=NCORES)

    qt_d = nc.declare_dram_parameter("qt", [D, QROWS], BF16, isOutput=False)
    kt_d = nc.declare_dram_parameter("kt", [D, S], BF16, isOutput=False)
    vt_d = nc.declare_dram_parameter("vt", [D, S], BF16, isOutput=False)
    wq_d = nc.declare_dram_parameter("wq", [D, DK], BF16, isOutput=False)
    wk_d = nc.declare_dram_parameter("wk", [D, DK], BF16, isOutput=False)
    wv_d = nc.declare_dram_parameter("wv", [D, DK], BF16, isOutput=False)
    if causal:
        mask_d = nc.declare_dram_parameter("maskq", [JB, 4 * CH], BF16,
                                           isOutput=False)
    ones_d = nc.declare_dram_parameter("ones", [128, 1], F32R, isOutput=False)
    # raw transposed accumulators (+denominator row); host normalizes
    out_d = nc.declare_dram_parameter("out", [NCH, DK + 1, CH], F32,
                                      isOutput=True)

    NSC = S // 512  # 8 column groups of 512 for k/v
    NSCQ = QROWS // 512  # 4 for q

    qt3 = qt_d.rearrange("(o p) s -> p o s", p=128)
    kt3 = kt_d.rearrange("(o p) s -> p o s", p=128)
    vt3 = vt_d.rearrange("(o p) s -> p o s", p=128)

    with tile.TileContext(nc) as tc:
        with (
            tc.tile_pool(name="const", bufs=1) as const,
            tc.tile_pool(name="res", bufs=1) as res,
            tc.tile_pool(name="stage", bufs=20) as stage,
            tc.tile_pool(name="pwork", bufs=4) as pwork,
            tc.tile_pool(name="ps_mm", bufs=2, space="PSUM") as ps_mm,
            tc.tile_pool(name="ps_s", bufs=2, space="PSUM") as ps_s,
            tc.tile_pool(name="ps_ot", bufs=2, space="PSUM") as ps_ot,
        ):
            def stage_load(src3, sc, splits=2):
                """Split-group DMAs so the first matmuls start early."""
                w = DC // splits
                sts = []
                for hh in range(splits):
                    st = stage.tile([128, w, 512], BF16, tag="stage",
                                    name=f"st{hh}")
                    nc.sync.dma_start(
                        st[:],
                        src3[:, w * hh:w * (hh + 1), sc * 512:(sc + 1) * 512])
                    sts.append(st)
                return sts

            def project_sc(src3, w_sb, sc, kind, sts=None):
                """One 512-column group: split DMAs + 8 accumulating
                matmuls; psum copied to the kT/qT tile."""
                if sts is None:
                    sts = stage_load(src3, sc)
                w = DC // len(sts)
                ps = ps_mm.tile([DK, 512], F32, tag="mm")
                for dc in range(DC):
                    nc.tensor.matmul(ps[:], w_sb[:, dc, :],
                                     sts[dc // w][:, dc % w, :],
                                     start=(dc == 0), stop=(dc == DC - 1))
                nc.vector.tensor_copy(
                    (kts if kind == "k" else qts)[sc][:], ps[:])

            def project_v(sc, sts=None):
                """V projected directly to natural [s, c] blocks: lhsT is the
                staged data chunk, rhs the weights -> out [128 s, 64 c], which
                is exactly the PV lhsT layout (no PE transposes needed)."""
                if sts is None:
                    sts = stage_load(vt3, sc)
                w = DC // len(sts)
                ps = ps_mm.tile([128, 4, DK], F32, tag="mm", name="ps_v")
                for sb in range(4):
                    for dc in range(DC):
                        nc.tensor.matmul(
                            ps[:, sb, :],
                            sts[dc // w][:, dc % w,
                                         sb * 128:(sb + 1) * 128],
                            wv_sb[:, dc, :],
                            start=(dc == 0), stop=(dc == DC - 1))
                for sb in range(4):
                    nc.vector.tensor_copy(vgs[sc][:, sb, 0:DK], ps[:, sb, :])
                nc.vector.tensor_copy(
                    vgs[sc][:, :, DK:DK + 1],
                    ones_sb[:].to_broadcast((128, 4, 1)))

            # PE warm-up in the initial DMA shadow: keeps the HAM clock at
            # full rate when the first real projections arrive
            warm = const.tile([128, 512], BF16, tag="warm")
            nc.vector.memset(warm[:], 0.0)
            for _ in range(8):
                wps = ps_mm.tile([DK, 512], F32, tag="mm", name="wps")
                nc.tensor.matmul(wps[:], warm[:, 0:DK], warm[:],
                                 start=True, stop=True)
            wq_sb = const.tile([128, DC, DK], BF16, tag="wq")
            wk_sb = const.tile([128, DC, DK], BF16, tag="wk")
            wv_sb = const.tile([128, DC, DK], BF16, tag="wv")
            head_q0 = stage_load(qt3, 0)
            ones_sb = const.tile([128, 1], F32R, tag="ones")
            nc.sync.dma_start(ones_sb[:], ones_d[:])
            nc.sync.dma_start(wq_sb[:], wq_d.rearrange("(o p) c -> p o c", p=128))
            nc.sync.dma_start(wk_sb[:], wk_d.rearrange("(o p) c -> p o c", p=128))
            nc.sync.dma_start(wv_sb[:], wv_d.rearrange("(o p) c -> p o c", p=128))
            head_k0 = stage_load(kt3, 0)
            head_v0 = stage_load(vt3, 0)
            if causal:
                msk_sb = const.tile([JB, 4 * CH], BF16, tag="msk")
                nc.sync.dma_start(msk_sb[:], mask_d[:])

            # per-chunk output accumulators in SBUF (row 64 = denominator)
            osb = res.tile([DK + 1, NCH, CH], F32, tag="osb")

            # kT tiles [64, 512] (4 key blocks per 512-col group)
            kts = [res.tile([DK, 512], F32R, tag=f"kt{sc}", name=f"kt{sc}")
                   for sc in range(NSC)]
            # qT tiles [64, 512]
            qts = [res.tile([DK, 512], F32R, tag=f"qt{sc}", name=f"qt{sc}")
                   for sc in range(NSCQ)]
            # v natural (+ones col): per 512-group, 4 blocks of [128, 65]
            vgs = [res.tile([128, 4, DK + 1], F32R, tag=f"vg{sc}",
                            name=f"vg{sc}")
                   for sc in range(NSC)]

            def v_lhsT(j):
                return vgs[j // 4][:, j % 4, :]

            def q_rhs(g):
                return qts[g // 2][:, (g % 2) * CH:(g % 2 + 1) * CH]

            pending = []  # [(g, t, p_tile)] awaiting PV + accumulate

            def emit_pv(item):
                g, t, p_sb = item
                o_tmp = ps_ot.tile([DK + 1, CH], F32, tag="ot", name="o_tmp")
                for u in range(4):
                    j = 4 * t + u
                    nc.tensor.matmul(
                        o_tmp[:], v_lhsT(j), p_sb[:, u * CH:(u + 1) * CH],
                        start=(u == 0), stop=(u == 3))
                if t == 0:
                    nc.vector.tensor_copy(osb[:, g, :], o_tmp[:])
                else:
                    nc.vector.tensor_add(osb[:, g, :], osb[:, g, :], o_tmp[:])

            def drain(upto):
                while len(pending) > upto:
                    emit_pv(pending.pop(0))

            def quad_block(g, t):
                s_ps = ps_s.tile([JB, 4 * CH], F32, tag="s", name="s_ps")
                for u in range(4):
                    j = 4 * t + u
                    nc.tensor.matmul(
                        s_ps[:, u * CH:(u + 1) * CH],
                        kts[j // 4][:, (j % 4) * JB:(j % 4 + 1) * JB],
                        q_rhs(g), start=True, stop=True)
                p_sb = pwork.tile([JB, 4 * CH], F32R, tag="p")
                finale = causal and g == NCH - 1 and t == nq[g] - 1
                if finale:
                    # last chunk's boundary quad is the closing serial chain:
                    # halve exp+mask so the first PVs start ~0.8us earlier
                    for hh in range(2):
                        sl = slice(hh * 2 * CH, (hh + 1) * 2 * CH)
                        nc.scalar.activation(
                            p_sb[:, sl], s_ps[:, sl],
                            mybir.ActivationFunctionType.Exp, scale=0.125)
                        nc.vector.tensor_mul(p_sb[:, sl], p_sb[:, sl],
                                             msk_sb[:, sl])
                else:
                    nc.scalar.activation(p_sb[:], s_ps[:],
                                         mybir.ActivationFunctionType.Exp,
                                         scale=0.125)
                    if causal and t == nq[g] - 1:
                        nc.vector.tensor_mul(p_sb[:], p_sb[:], msk_sb[:])
                pending.append((g, t, p_sb))
                drain(2)

            def epilogue(g):
                # POOL DGE so result stores don't head-of-line block the SP
                # sequencer issuing input stage loads; the last two chunks go
                # via the faster HWDGE since all input loads are done by then
                eng = nc.sync if g >= NCH - 2 else nc.gpsimd
                eng.dma_start(out_d[g], osb[:, g, :])

            # key-major sweep; next key group's projections are interleaved
            # into the current step's quads so the in-order PE never idles at
            # step boundaries
            project_sc(qt3, wq_sb, 0, "q", sts=head_q0)
            project_sc(kt3, wk_sb, 0, "k", sts=head_k0)
            project_v(0, sts=head_v0)
            for t in range(NSC):
                todo = ([("k", t + 1), ("v", t + 1)] if t + 1 < NSC else [])
                chunks = [g for g in range(NCH) if t < nq[g]]
                for g in chunks:
                    if t == 0 and g > 0 and g % 2 == 0:
                        project_sc(qt3, wq_sb, g // 2, "q")
                    quad_block(g, t)
                    if t == nq[g] - 1:
                        drain(0)
                        epilogue(g)
                # next key group's projections emitted at step end: the
                # scheduler overlaps their DMA-paced matmuls with this step's
                # attention without blocking the in-order PE mid-step
                for kind, sc in todo:
                    if kind == "v":
                        project_v(sc)
                    else:
                        project_sc(kt3, wk_sb, sc, kind)
                drain(0)

    nc.compile()
    return nc


def _get_prog(variant):
    if variant not in _prog_cache:
        _prog_cache[variant] = _build(variant)
    return _prog_cache[variant]


def _mask_quad(h):
    """Multiplicative boundary mask [JB, 4*CH] for the final key quad of every
    chunk of core half h: block m of the quad allows (i - j) >= 128*m - 256*h."""
    i = np.arange(CH)[None, :]
    j = np.arange(JB)[:, None]
    tiles = [((i - j) >= (128 * m - 256 * h)).astype(np.float32)
             for m in range(4)]
    return np.concatenate(tiles, axis=1)


def kernel(queries, keys, values, Wq, Wk, Wv, mask):
    import ml_dtypes  # noqa: F401  registers numpy bfloat16

    bf16 = np.dtype("bfloat16")
    queries = np.asarray(queries, dtype=np.float32)
    keys = np.asarray(keys, dtype=np.float32)
    values = np.asarray(values, dtype=np.float32)
    mask_np = np.asarray(mask)

    causal = bool(np.array_equal(
        mask_np != 0, np.tril(np.ones((S, S), dtype=bool))))
    full = bool((mask_np != 0).all()) if not causal else False
    if not (causal or full):
        raise NotImplementedError("general mask not supported")
    variant = "causal" if causal else "full"

    qt = np.ascontiguousarray(queries.transpose(0, 2, 1)).astype(bf16)
    kt = np.ascontiguousarray(keys.transpose(0, 2, 1)).astype(bf16)
    vt = np.ascontiguousarray(values.transpose(0, 2, 1)).astype(bf16)
    wq = np.ascontiguousarray(np.asarray(Wq, dtype=np.float32).T).astype(bf16)
    wk = np.ascontiguousarray(np.asarray(Wk, dtype=np.float32).T).astype(bf16)
    wv = np.ascontiguousarray(np.asarray(Wv, dtype=np.float32).T).astype(bf16)

    in_maps = []
    for core in range(NCORES):
        b, h = divmod(core, 2)
        qsel = np.ascontiguousarray(
            qt[b].reshape(D, 2 * NCH, CH)[:, h::2, :].reshape(D, QROWS))
        m = {"qt": qsel, "kt": kt[b], "vt": vt[b],
             "wq": wq, "wk": wk, "wv": wv,
             "ones": np.ones((128, 1), dtype=np.float32)}
        if variant == "causal":
            m["maskq"] = _mask_quad(h).astype(bf16)
        in_maps.append(m)

    global _last_in_maps
    _last_in_maps = in_maps
    nc = _get_prog(variant)
    res = run_bass_kernel_spmd(nc, in_maps, list(range(NCORES)))

    out = np.empty((B, S, DK), dtype=np.float32)
    ov = out.reshape(B, 2 * NCH, CH, DK)
    for core in range(NCORES):
        b, h = divmod(core, 2)
        raw = res.results[core]["out"]  # [NCH, DK+1, CH]
        ov[b, h::2] = (raw[:, :DK, :] / raw[:, DK:DK + 1, :]).transpose(0, 2, 1)
    return out


if __name__ == "__main__":
    rng = np.random.default_rng(0)
    q = rng.standard_normal((B, S, D), dtype=np.float32)
    k = rng.standard_normal((B, S, D), dtype=np.float32)
    v = rng.standard_normal((B, S, D), dtype=np.float32)
    sc = 1.0 / np.sqrt(D)
    wq = rng.uniform(-sc, sc, (DK, D)).astype(np.float32)
    wk = rng.uniform(-sc, sc, (DK, D)).astype(np.float32)
    wv = rng.uniform(-sc, sc, (DK, D)).astype(np.float32)
    msk = np.tril(np.ones((S, S), dtype=np.int32))
    out = kernel(queries=q, keys=k, values=v, Wq=wq, Wk=wk, Wv=wv, mask=msk)
    print("out", out.shape, out.dtype, float(np.abs(out).mean()))



# revision 3
# speedup vs baseline: 1.0139x; 1.0139x over previous
"""Causal single-head attention (B=4, S=4096, D=1024, d_key=64) on 8 trn2 cores.

Sharding: 8 cores = 4 batches x 2 key-halves. Core (b, h) handles batch b,
ALL 4096 query rows, and the 16 alternating 128-key blocks {2j+h : j=0..15}.
Each core computes partial PV numerators and softmax denominators over its
key half; the host merges the two halves per batch:
    out = (num_0 + num_1) / (den_0 + den_1).

This halves the K/V HBM traffic per core vs replicating K/V on both cores
of a pair (17.8MB vs 20.6MB) and makes the two cores of a pair perfectly
symmetric (identical work; only the boundary mask data differs by h).

Device kernel (identical SPMD program; per-core differences are input data):
  1. Projections (bf16 matmuls, fp32 accumulate, bf16 results):
     kT [64, 2048] over own key blocks, qT [64, 4096] over all rows, and
     v-natural [128, 65] blocks (data chunk as lhsT -> PV lhsT layout
     directly; column 64 is memset to 1.0 so PV also emits the softmax
     denominator row).
  2. QUERY-MAJOR attention: for each 256-row query chunk c (0..15), for each
     local key quad t <= c//4 (quad = 4 own 128-key blocks = 512 keys):
     scores sT[j, i] = k_j . q_i into a [128, <=1024] PSUM strip, one ACT
     exp (scale=1/8, bf16 out), causal boundary mask multiply on the
     diagonal block only (host-built [128, 256] bf16 triangle, offset by h),
     then PV-accumulates all of chunk c's quads into one PSUM [65, 256]
     chain. Per chunk: copy to bf16 SBUF and DMA out (gpsimd DGE so stores
     don't block input stage loads on the SP sequencer).
  3. Input DMA order is chosen so the LAST arrivals (k-quad 3, v-quad 3)
     gate only ~2.5us of tail work (boundary quads of chunks 12-15), while
     all 8 q-groups land before them.
"""

import numpy as np

import concourse.mybir as mybir
import concourse.tile as tile
from concourse import bacc
from concourse.bass_utils import run_bass_kernel_spmd

B, S, D, DK = 4, 4096, 1024, 64
NCORES = 8
CH = 256  # query rows per chunk
NCH = 16  # chunks per core (all 4096 rows)
KB = 128  # key block
NKB = 16  # own key blocks per core (half of 32)
NKQ = 4  # own key quads (4 blocks of 128 = 512 keys each)
DC = D // 128  # 8 contraction chunks
F32 = mybir.dt.float32
BF16 = mybir.dt.bfloat16

_prog_cache = {}
_last_in_maps = None


def _build(variant):
    assert variant == "causal"

    nc = bacc.Bacc("TRN2", target_bir_lowering=False, debug=False,
                   num_devices=NCORES)

    qt_d = nc.declare_dram_parameter("qt", [D, S], BF16, isOutput=False)
    kt_d = nc.declare_dram_parameter("kt", [D, NKB * KB], BF16, isOutput=False)
    vt_d = nc.declare_dram_parameter("vt", [D, NKB * KB], BF16, isOutput=False)
    # weights packed host-side as [128, DC, DK] so each partition row is
    # contiguous (fast DMA)
    wq_d = nc.declare_dram_parameter("wq", [128, DC * DK], BF16, isOutput=False)
    wk_d = nc.declare_dram_parameter("wk", [128, DC * DK], BF16, isOutput=False)
    wv_d = nc.declare_dram_parameter("wv", [128, DC * DK], BF16, isOutput=False)
    mask_d = nc.declare_dram_parameter("maskq", [KB, CH], BF16, isOutput=False)
    # partial numerators (rows 0..63) + denominator (row 64), bf16
    out_d = nc.declare_dram_parameter("out", [NCH, DK + 1, CH], BF16,
                                      isOutput=True)

    qt3 = qt_d.rearrange("(o p) s -> p o s", p=128)
    kt3 = kt_d.rearrange("(o p) s -> p o s", p=128)
    vt3 = vt_d.rearrange("(o p) s -> p o s", p=128)

    NQG = S // 512  # 8 q projection groups of 512 columns
    NKG = NKQ  # 4 k/v projection groups of 512 local keys

    with tile.TileContext(nc) as tc:
        with (
            tc.tile_pool(name="const", bufs=1) as const,
            tc.tile_pool(name="res", bufs=1) as res,
            tc.tile_pool(name="stage", bufs=6) as stage,
            tc.tile_pool(name="pwork", bufs=5) as pwork,
            tc.tile_pool(name="ps_mm", bufs=2, space="PSUM") as ps_mm,
            tc.tile_pool(name="ps_s", bufs=2, space="PSUM") as ps_s,
            tc.tile_pool(name="ps_ot", bufs=2, space="PSUM") as ps_ot,
        ):
            def stage_load(src3, sc, splits=2):
                """Split-group DMAs so dependent matmuls can start early."""
                w = DC // splits
                sts = []
                for hh in range(splits):
                    st = stage.tile([128, w, 512], BF16, tag="stage",
                                    name=f"st{hh}")
                    nc.sync.dma_start(
                        st[:],
                        src3[:, w * hh:w * (hh + 1), sc * 512:(sc + 1) * 512])
                    sts.append(st)
                return sts

            def project_kq(w_sb, dst, sc, sts):
                """One 512-column group -> dst tile [64, 512] (bf16)."""
                w = DC // len(sts)
                ps = ps_mm.tile([DK, 512], F32, tag="mm")
                for dc in range(DC):
                    nc.tensor.matmul(ps[:], w_sb[:, dc, :],
                                     sts[dc // w][:, dc % w, :],
                                     start=(dc == 0), stop=(dc == DC - 1))
                nc.vector.tensor_copy(dst[:], ps[:])

            def project_v(sc, sts):
                """V projected directly to natural [s, c] blocks: lhsT is the
                staged data chunk, rhs the weights -> out [128 s, 64 c], the
                PV lhsT layout (no transposes)."""
                w = DC // len(sts)
                ps = ps_mm.tile([128, 4, DK], F32, tag="mm", name="ps_v")
                for sb in range(4):
                    for dc in range(DC):
                        nc.tensor.matmul(
                            ps[:, sb, :],
                            sts[dc // w][:, dc % w,
                                         sb * 128:(sb + 1) * 128],
                            wv_sb[:, dc, :],
                            start=(dc == 0), stop=(dc == DC - 1))
                for sb in range(4):
                    nc.vector.tensor_copy(vgs[sc][:, sb, 0:DK], ps[:, sb, :])

            # PE warm-up in the initial DMA shadow: keeps the HAM clock at
            # full rate when the first real projections arrive
            warm = const.tile([128, 512], BF16, tag="warm")
            nc.vector.memset(warm[:], 0.0)
            for _ in range(8):
                wps = ps_mm.tile([DK, 512], F32, tag="mm", name="wps")
                nc.tensor.matmul(wps[:], warm[:, 0:DK], warm[:],
                                 start=True, stop=True)

            wq_sb = const.tile([128, DC, DK], BF16, tag="wq")
            wk_sb = const.tile([128, DC, DK], BF16, tag="wk")
            wv_sb = const.tile([128, DC, DK], BF16, tag="wv")
            nc.sync.dma_start(wq_sb[:], wq_d.rearrange("p (o c) -> p o c", c=DK))
            nc.sync.dma_start(wk_sb[:], wk_d.rearrange("p (o c) -> p o c", c=DK))
            nc.sync.dma_start(wv_sb[:], wv_d.rearrange("p (o c) -> p o c", c=DK))
            msk_sb = const.tile([KB, CH], BF16, tag="msk")
            nc.sync.dma_start(msk_sb[:], mask_d[:])

            # kT tiles [64, 512] per local key quad (bf16)
            kts = [res.tile([DK, 512], BF16, tag=f"kt{t}", name=f"kt{t}")
                   for t in range(NKQ)]
            # qT tiles [64, 512] per q group (bf16)
            qts = [res.tile([DK, 512], BF16, tag=f"qt{g}", name=f"qt{g}")
                   for g in range(NQG)]
            # v natural (+ones col): per quad, 4 blocks of [128, 65] (bf16)
            vgs = [res.tile([128, 4, DK + 1], BF16, tag=f"vg{t}",
                            name=f"vg{t}")
                   for t in range(NKQ)]
            for t in range(NKQ):
                nc.vector.memset(vgs[t][:, :, DK:DK + 1], 1.0)

            # bf16 output bounce (PSUM -> SBUF -> DRAM)
            osb = res.tile([DK + 1, NCH, CH], BF16, tag="osb")

            # Input DMA order: k0 q0 v0 q1 q2 k1 v1 q3 q4 k2 v2 q5 q6 q7 k3 v3
            # (all q groups land before the last k/v quads so the post-DMA
            # tail is only the boundary quads of chunks 12-15).
            load_order = [("k", 0), ("q", 0), ("v", 0), ("q", 1), ("q", 2),
                          ("k", 1), ("v", 1), ("q", 3), ("q", 4), ("k", 2),
                          ("v", 2), ("q", 5), ("q", 6), ("q", 7), ("k", 3),
                          ("v", 3)]
            staged = {}
            emitted = []

            def ensure_loaded(upto):
                """Emit stage loads in order through index `upto`."""
                for i in range(len(emitted), upto + 1):
                    kind, idx = load_order[i]
                    src = {"k": kt3, "q": qt3, "v": vt3}[kind]
                    staged[(kind, idx)] = stage_load(src, idx)
                    emitted.append((kind, idx))

            def q_rhs(c):
                return qts[c // 2][:, (c % 2) * CH:(c % 2 + 1) * CH]

            projected = set()

            def ensure_projected(kind, idx):
                if (kind, idx) in projected:
                    return
                projected.add((kind, idx))
                i = load_order.index((kind, idx))
                ensure_loaded(i)
                sts = staged.pop((kind, idx))
                if kind == "q":
                    project_kq(wq_sb, qts[idx], idx, sts)
                elif kind == "k":
                    project_kq(wk_sb, kts[idx], idx, sts)
                else:
                    project_v(idx, sts)

            # prefetch schedule: keep a few loads in flight ahead of compute
            def prefetch(upto):
                ensure_loaded(min(upto, len(load_order) - 1))

            # Strip pipeline with lag: emit scores+exp for strip N+LAG before
            # the PV chain of strip N, so the in-order PE never waits on the
            # ACT exp of the strip it just produced.
            pending = []  # dicts awaiting PV emission

            def emit_pv(item):
                for u in range(item["nb"]):
                    nc.tensor.matmul(
                        item["o_ps"][:], vgs[item["t"]][:, u, :],
                        item["p"][:, u * CH:(u + 1) * CH],
                        start=(item["start"] and u == 0),
                        stop=(item["stop"] and u == item["nb"] - 1))
                close = item.get("close")
                if close is not None:
                    mode, c, eng = close
                    if mode == "copy":
                        nc.vector.tensor_copy(osb[:, c, :], item["o_ps"][:])
                    else:
                        nc.vector.tensor_add(osb[:, c, :], osb[:, c, :],
                                             item["o_ps"][:])
                    if eng is not None:
                        eng.dma_start(out_d[c], osb[:, c, :])

            def drain(upto):
                while len(pending) > upto:
                    emit_pv(pending.pop(0))

            def strip(c, t, nb, o_ps, start, stop, close):
                """Scores + exp (+ boundary mask) for quad t of chunk c,
                covering its first nb key blocks."""
                ensure_projected("k", t)
                ncols = nb * CH
                s_ps = ps_s.tile([KB, 4 * CH], F32, tag="s", name="s_ps")
                for u in range(nb):
                    nc.tensor.matmul(
                        s_ps[:, u * CH:(u + 1) * CH],
                        kts[t][:, u * KB:(u + 1) * KB],
                        q_rhs(c), start=True, stop=True)
                p_sb = pwork.tile([KB, 4 * CH], BF16, tag="p")
                nc.scalar.activation(
                    p_sb[:, 0:ncols], s_ps[:, 0:ncols],
                    mybir.ActivationFunctionType.Exp, scale=0.125)
                if t == c // 4:
                    # causal boundary: diagonal block is the last one
                    sl = slice((nb - 1) * CH, nb * CH)
                    nc.vector.tensor_mul(p_sb[:, sl], p_sb[:, sl], msk_sb[:])
                ensure_projected("v", t)
                pending.append(dict(c=c, t=t, nb=nb, p=p_sb, o_ps=o_ps,
                                    start=start, stop=stop, close=close))
                drain(2)

            # Chunks 0-11 need only k/v quads 0-2, which arrive early; their
            # boundary quad joins the single per-chunk PV chain. Chunks 12-15
            # split off their boundary quad (k3/v3, the last DMA arrivals)
            # into a tail pass so nothing queues behind those loads.
            for c in range(NCH):
                ensure_projected("q", c // 2)
                bq = c // 4  # boundary quad index
                o_ps = ps_ot.tile([DK + 1, CH], F32, tag="ot", name="o_ps")
                if c < 12:
                    for t in range(bq + 1):
                        nb = 4 if t < bq else c % 4 + 1
                        strip(c, t, nb, o_ps, start=(t == 0), stop=(t == bq),
                              close=("copy", c, nc.gpsimd) if t == bq else None)
                else:
                    for t in range(3):
                        strip(c, t, 4, o_ps, start=(t == 0), stop=(t == 2),
                              close=("copy", c, None) if t == 2 else None)
                # keep input loads running ahead of compute
                prefetch(2 * (c // 2) + 4)
            # tail: boundary quads of chunks 12-15 (quad 3 = last k/v loads)
            for c in range(12, NCH):
                o_ps = ps_ot.tile([DK + 1, CH], F32, tag="ot", name="o_ps2")
                strip(c, 3, c % 4 + 1, o_ps, start=True, stop=True,
                      close=("add", c, nc.sync))
            drain(0)

    nc.compile()
    return nc


def _get_prog(variant):
    if variant not in _prog_cache:
        _prog_cache[variant] = _build(variant)
    return _prog_cache[variant]


def _mask_block(h):
    """Multiplicative boundary mask [KB, CH] for the diagonal own-block of
    every chunk of core-half h: local key row kappa (global key 256c + 128h
    + kappa) allows query column i (global row 256c + i) iff
    i >= kappa + 128h."""
    i = np.arange(CH)[None, :]
    kap = np.arange(KB)[:, None]
    return (i >= kap + 128 * h).astype(np.float32)


def kernel(queries, keys, values, Wq, Wk, Wv, mask):
    import ml_dtypes  # noqa: F401  registers numpy bfloat16

    bf16 = np.dtype("bfloat16")
    queries = np.asarray(queries, dtype=np.float32)
    keys = np.asarray(keys, dtype=np.float32)
    values = np.asarray(values, dtype=np.float32)
    mask_np = np.asarray(mask)

    causal = bool(np.array_equal(
        mask_np != 0, np.tril(np.ones((S, S), dtype=bool))))
    if not causal:
        raise NotImplementedError("only the causal mask is supported")

    qt = np.ascontiguousarray(queries.transpose(0, 2, 1)).astype(bf16)
    # per-batch [D, S] -> own-key-half [D, 2048] with alternating 128 blocks
    kt_f = np.asarray(keys, dtype=np.float32).transpose(0, 2, 1)
    vt_f = np.asarray(values, dtype=np.float32).transpose(0, 2, 1)
    kt_blk = kt_f.reshape(B, D, S // KB, KB)
    vt_blk = vt_f.reshape(B, D, S // KB, KB)

    def pack_w(W):
        # [DK, D] -> [128, DC*DK] with w[p, dc*DK+k] = W[k, dc*128+p]
        Wt = np.asarray(W, dtype=np.float32).T.reshape(DC, 128, DK)
        return np.ascontiguousarray(Wt.transpose(1, 0, 2).reshape(128, DC * DK)
                                    ).astype(bf16)

    wq, wk, wv = pack_w(Wq), pack_w(Wk), pack_w(Wv)

    in_maps = []
    for core in range(NCORES):
        b, h = divmod(core, 2)
        kth = np.ascontiguousarray(
            kt_blk[b, :, h::2, :].reshape(D, NKB * KB)).astype(bf16)
        vth = np.ascontiguousarray(
            vt_blk[b, :, h::2, :].reshape(D, NKB * KB)).astype(bf16)
        m = {"qt": qt[b], "kt": kth, "vt": vth,
             "wq": wq, "wk": wk, "wv": wv,
             "maskq": _mask_block(h).astype(bf16)}
        in_maps.append(m)

    global _last_in_maps
    _last_in_maps = in_maps
    nc = _get_prog("causal")
    res = run_bass_kernel_spmd(nc, in_maps, list(range(NCORES)))

    out = np.empty((B, S, DK), dtype=np.float32)
    ov = out.reshape(B, NCH, CH, DK)
    for b in range(B):
        r0 = np.asarray(res.results[2 * b]["out"], dtype=np.float32)
        r1 = np.asarray(res.results[2 * b + 1]["out"], dtype=np.float32)
        num = r0[:, :DK, :] + r1[:, :DK, :]  # [NCH, DK, CH]
        den = r0[:, DK:DK + 1, :] + r1[:, DK:DK + 1, :]  # [NCH, 1, CH]
        ov[b] = (num / den).transpose(0, 2, 1)
    return out


if __name__ == "__main__":
    rng = np.random.default_rng(0)
    q = rng.standard_normal((B, S, D), dtype=np.float32)
    k = rng.standard_normal((B, S, D), dtype=np.float32)
    v = rng.standard_normal((B, S, D), dtype=np.float32)
    sc = 1.0 / np.sqrt(D)
    wq = rng.uniform(-sc, sc, (DK, D)).astype(np.float32)
    wk = rng.uniform(-sc, sc, (DK, D)).astype(np.float32)
    wv = rng.uniform(-sc, sc, (DK, D)).astype(np.float32)
    msk = np.tril(np.ones((S, S), dtype=np.int32))
    out = kernel(queries=q, keys=k, values=v, Wq=wq, Wk=wk, Wv=wv, mask=msk)
    print("out", out.shape, out.dtype, float(np.abs(out).mean()))


# revision 7
# speedup vs baseline: 1.0737x; 1.0589x over previous
"""Causal single-head attention (B=4, S=4096, D=1024, d_key=64) on 8 trn2 cores.

Sharding: 8 cores = 4 batches x 2 key-halves. Core (b, h) handles batch b,
ALL 4096 query rows, and the 16 alternating 128-key blocks {2j+h : j=0..15}.
Each core computes partial PV numerators and softmax denominators over its
key half; the host merges the two halves per batch:
    out = (num_0 + num_1) / (den_0 + den_1).

This halves the K/V HBM traffic per core vs replicating K/V on both cores
of a pair (17.8MB vs 20.6MB) and makes the two cores of a pair perfectly
symmetric (identical work; only the boundary mask data differs by h).

Device kernel (identical SPMD program; per-core differences are input data):
  1. Projections (bf16 matmuls, fp32 accumulate, bf16 results):
     kT [64, 2048] over own key blocks, qT [64, 4096] over all rows, and
     v-natural [128, 65] blocks (data chunk as lhsT -> PV lhsT layout
     directly; column 64 is memset to 1.0 so PV also emits the softmax
     denominator row).
  2. QUERY-MAJOR attention: for each 256-row query chunk c (0..15), for each
     local key quad t <= c//4 (quad = 4 own 128-key blocks = 512 keys):
     scores sT[j, i] = k_j . q_i into a [128, <=1024] PSUM strip, one ACT
     exp (scale=1/8, bf16 out), causal boundary mask multiply on the
     diagonal block only (host-built [128, 256] bf16 triangle, offset by h),
     then PV-accumulates all of chunk c's quads into one PSUM [65, 256]
     chain. Per chunk: copy to bf16 SBUF and DMA out (gpsimd DGE so stores
     don't block input stage loads on the SP sequencer).
  3. Input DMA order is chosen so the LAST arrivals (k-quad 3, v-quad 3)
     gate only ~2.5us of tail work (boundary quads of chunks 12-15), while
     all 8 q-groups land before them.
"""

import numpy as np

import concourse.mybir as mybir
import concourse.tile as tile
from concourse import bacc
from concourse.bass_utils import run_bass_kernel_spmd

B, S, D, DK = 4, 4096, 1024, 64
NCORES = 8
CH = 256  # query rows per chunk
NCH = 16  # chunks per core (all 4096 rows)
KB = 128  # key block
NKB = 16  # own key blocks per core (half of 32)
NKQ = 4  # own key quads (4 blocks of 128 = 512 keys each)
DC = D // 128  # 8 contraction chunks
F32 = mybir.dt.float32
BF16 = mybir.dt.bfloat16

_prog_cache = {}
_last_in_maps = None


def _build(variant):
    assert variant == "causal"

    nc = bacc.Bacc("TRN2", target_bir_lowering=False, debug=False,
                   num_devices=NCORES)

    qt_d = nc.declare_dram_parameter("qt", [D, S], BF16, isOutput=False)
    kt_d = nc.declare_dram_parameter("kt", [D, NKB * KB], BF16, isOutput=False)
    vt_d = nc.declare_dram_parameter("vt", [D, NKB * KB], BF16, isOutput=False)
    # weights packed host-side as [128, DC, DK] so each partition row is
    # contiguous (fast DMA)
    wq_d = nc.declare_dram_parameter("wq", [128, DC * DK], BF16, isOutput=False)
    wk_d = nc.declare_dram_parameter("wk", [128, DC * DK], BF16, isOutput=False)
    wv_d = nc.declare_dram_parameter("wv", [128, DC * DK], BF16, isOutput=False)
    mask_d = nc.declare_dram_parameter("maskq", [KB, CH], BF16, isOutput=False)
    # partial numerators (rows 0..63) + denominator (row 64), bf16
    out_d = nc.declare_dram_parameter("out", [NCH, DK + 1, CH], BF16,
                                      isOutput=True)

    qt3 = qt_d.rearrange("(o p) s -> p o s", p=128)
    kt3 = kt_d.rearrange("(o p) s -> p o s", p=128)
    vt3 = vt_d.rearrange("(o p) s -> p o s", p=128)

    NQG = S // 512  # 8 q projection groups of 512 columns
    NKG = NKQ  # 4 k/v projection groups of 512 local keys

    with tile.TileContext(nc) as tc:
        with (
            tc.tile_pool(name="const", bufs=1) as const,
            tc.tile_pool(name="res", bufs=1) as res,
            tc.tile_pool(name="stage", bufs=6) as stage,
            tc.tile_pool(name="pwork", bufs=5) as pwork,
            tc.tile_pool(name="ps_mm", bufs=2, space="PSUM") as ps_mm,
            tc.tile_pool(name="ps_s", bufs=2, space="PSUM") as ps_s,
            tc.tile_pool(name="ps_ot", bufs=2, space="PSUM") as ps_ot,
        ):
            def stage_load(src3, sc, splits=2):
                """Split-group DMAs so dependent matmuls can start early."""
                w = DC // splits
                sts = []
                for hh in range(splits):
                    st = stage.tile([128, w, 512], BF16, tag="stage",
                                    name=f"st{hh}")
                    nc.sync.dma_start(
                        st[:],
                        src3[:, w * hh:w * (hh + 1), sc * 512:(sc + 1) * 512])
                    sts.append(st)
                return sts

            def project_kq(w_sb, dst, sc, sts):
                """One 512-column group -> dst tile [64, 512] (bf16)."""
                w = DC // len(sts)
                ps = ps_mm.tile([DK, 512], F32, tag="mm")
                for dc in range(DC):
                    nc.tensor.matmul(ps[:], w_sb[:, dc, :],
                                     sts[dc // w][:, dc % w, :],
                                     start=(dc == 0), stop=(dc == DC - 1))
                nc.vector.tensor_copy(dst[:], ps[:])

            def project_v(sc, sts):
                """V projected directly to natural [s, c] blocks: lhsT is the
                staged data chunk, rhs the weights -> out [128 s, 64 c], the
                PV lhsT layout (no transposes)."""
                w = DC // len(sts)
                ps = ps_mm.tile([128, 4, DK], F32, tag="mm", name="ps_v")
                for sb in range(4):
                    for dc in range(DC):
                        nc.tensor.matmul(
                            ps[:, sb, :],
                            sts[dc // w][:, dc % w,
                                         sb * 128:(sb + 1) * 128],
                            wv_sb[:, dc, :],
                            start=(dc == 0), stop=(dc == DC - 1))
                for sb in range(4):
                    nc.vector.tensor_copy(vgs[sc][:, sb, 0:DK], ps[:, sb, :])

            # PE warm-up in the initial DMA shadow: keeps the HAM clock at
            # full rate when the first real projections arrive
            warm = const.tile([128, 512], BF16, tag="warm")
            nc.vector.memset(warm[:], 0.0)
            for _ in range(8):
                wps = ps_mm.tile([DK, 512], F32, tag="mm", name="wps")
                nc.tensor.matmul(wps[:], warm[:, 0:DK], warm[:],
                                 start=True, stop=True)

            wq_sb = const.tile([128, DC, DK], BF16, tag="wq")
            wk_sb = const.tile([128, DC, DK], BF16, tag="wk")
            wv_sb = const.tile([128, DC, DK], BF16, tag="wv")
            nc.sync.dma_start(wq_sb[:], wq_d.rearrange("p (o c) -> p o c", c=DK))
            nc.sync.dma_start(wk_sb[:], wk_d.rearrange("p (o c) -> p o c", c=DK))
            nc.sync.dma_start(wv_sb[:], wv_d.rearrange("p (o c) -> p o c", c=DK))
            msk_sb = const.tile([KB, CH], BF16, tag="msk")
            nc.sync.dma_start(msk_sb[:], mask_d[:])

            # kT tiles [64, 512] per local key quad (bf16)
            kts = [res.tile([DK, 512], BF16, tag=f"kt{t}", name=f"kt{t}")
                   for t in range(NKQ)]
            # qT tiles [64, 512] per q group (bf16)
            qts = [res.tile([DK, 512], BF16, tag=f"qt{g}", name=f"qt{g}")
                   for g in range(NQG)]
            # v natural (+ones col): per quad, 4 blocks of [128, 65] (bf16)
            vgs = [res.tile([128, 4, DK + 1], BF16, tag=f"vg{t}",
                            name=f"vg{t}")
                   for t in range(NKQ)]
            for t in range(NKQ):
                nc.vector.memset(vgs[t][:, :, DK:DK + 1], 1.0)

            # bf16 output bounce (PSUM -> SBUF -> DRAM)
            osb = res.tile([DK + 1, NCH, CH], BF16, tag="osb")

            # Input DMA order: the key-major sweep over quad t unlocks
            # (16-4t) chunks of attention, so work-per-arrival decreases over
            # the stream; q groups trickle in under sweep 0.
            load_order = [("k", 0), ("q", 0), ("v", 0), ("q", 1), ("q", 2),
                          ("q", 3), ("q", 4), ("q", 5), ("q", 6), ("q", 7),
                          ("k", 1), ("v", 1), ("k", 2), ("v", 2), ("k", 3),
                          ("v", 3)]
            staged = {}
            emitted = []

            def ensure_loaded(upto):
                """Emit stage loads in order through index `upto`."""
                for i in range(len(emitted), upto + 1):
                    kind, idx = load_order[i]
                    src = {"k": kt3, "q": qt3, "v": vt3}[kind]
                    staged[(kind, idx)] = stage_load(src, idx)
                    emitted.append((kind, idx))

            def q_rhs(c):
                return qts[c // 2][:, (c % 2) * CH:(c % 2 + 1) * CH]

            projected = set()

            def ensure_projected(kind, idx):
                if (kind, idx) in projected:
                    return
                projected.add((kind, idx))
                i = load_order.index((kind, idx))
                ensure_loaded(i)
                sts = staged.pop((kind, idx))
                if kind == "q":
                    project_kq(wq_sb, qts[idx], idx, sts)
                elif kind == "k":
                    project_kq(wk_sb, kts[idx], idx, sts)
                else:
                    project_v(idx, sts)

            # prefetch schedule: keep a few loads in flight ahead of compute
            def prefetch(upto):
                ensure_loaded(min(upto, len(load_order) - 1))

            # Strip pipeline with lag: emit scores+exp for strip N+LAG before
            # the PV chain of strip N, so the in-order PE never waits on the
            # ACT exp of the strip it just produced.
            pending = []  # dicts awaiting PV emission

            def emit_pv(item):
                c, t, nb = item["c"], item["t"], item["nb"]
                ensure_projected("v", t)
                o_ps = ps_ot.tile([DK + 1, CH], F32, tag="ot", name="o_ps")
                for u in range(nb):
                    nc.tensor.matmul(
                        o_ps[:], vgs[t][:, u, :],
                        item["p"][:, u * CH:(u + 1) * CH],
                        start=(u == 0), stop=(u == nb - 1))
                if t == 0:
                    nc.vector.tensor_copy(osb[:, c, :], o_ps[:])
                else:
                    nc.vector.tensor_add(osb[:, c, :], osb[:, c, :], o_ps[:])
                if t == c // 4:  # chunk finished: store partials
                    eng = nc.sync if t == NKQ - 1 else nc.gpsimd
                    eng.dma_start(out_d[c], osb[:, c, :])

            def drain(upto):
                while len(pending) > upto:
                    emit_pv(pending.pop(0))

            def strip(c, t):
                """Scores + exp (+ boundary mask) for quad t of chunk c."""
                nb = 4 if t < c // 4 else c % 4 + 1
                ncols = nb * CH
                s_ps = ps_s.tile([KB, 4 * CH], F32, tag="s", name="s_ps")
                for u in range(nb):
                    nc.tensor.matmul(
                        s_ps[:, u * CH:(u + 1) * CH],
                        kts[t][:, u * KB:(u + 1) * KB],
                        q_rhs(c), start=True, stop=True)
                p_sb = pwork.tile([KB, 4 * CH], BF16, tag="p")
                nc.scalar.activation(
                    p_sb[:, 0:ncols], s_ps[:, 0:ncols],
                    mybir.ActivationFunctionType.Exp, scale=0.125)
                if t == c // 4:
                    # causal boundary: diagonal block is the last one
                    sl = slice((nb - 1) * CH, nb * CH)
                    nc.vector.tensor_mul(p_sb[:, sl], p_sb[:, sl], msk_sb[:])
                pending.append(dict(c=c, t=t, nb=nb, p=p_sb))
                drain(2)

            # Key-major sweeps: quad t serves chunks 4t..15, so the work
            # unlocked by each k/v arrival shrinks over the stream. Sweep 0
            # interleaves the q-group projections as their loads land.
            for t in range(NKQ):
                if t > 0:
                    # flush pending PVs before the PE stalls on the next
                    # k-quad projection (their inputs are already on chip)
                    drain(0)
                ensure_projected("k", t)
                for c in range(4 * t, NCH):
                    if t == 0:
                        ensure_projected("q", c // 2)
                        prefetch(c + 2)
                    strip(c, t)
            drain(0)

    nc.compile()
    return nc


def _get_prog(variant):
    if variant not in _prog_cache:
        _prog_cache[variant] = _build(variant)
    return _prog_cache[variant]


def _mask_block(h):
    """Multiplicative boundary mask [KB, CH] for the diagonal own-block of
    every chunk of core-half h: local key row kappa (global key 256c + 128h
    + kappa) allows query column i (global row 256c + i) iff
    i >= kappa + 128h."""
    i = np.arange(CH)[None, :]
    kap = np.arange(KB)[:, None]
    return (i >= kap + 128 * h).astype(np.float32)


def kernel(queries, keys, values, Wq, Wk, Wv, mask):
    import ml_dtypes  # noqa: F401  registers numpy bfloat16

    bf16 = np.dtype("bfloat16")
    queries = np.asarray(queries, dtype=np.float32)
    keys = np.asarray(keys, dtype=np.float32)
    values = np.asarray(values, dtype=np.float32)
    mask_np = np.asarray(mask)

    causal = bool(np.array_equal(
        mask_np != 0, np.tril(np.ones((S, S), dtype=bool))))
    if not causal:
        raise NotImplementedError("only the causal mask is supported")

    qt = np.ascontiguousarray(queries.transpose(0, 2, 1)).astype(bf16)
    # per-batch [D, S] -> own-key-half [D, 2048] with alternating 128 blocks
    kt_f = np.asarray(keys, dtype=np.float32).transpose(0, 2, 1)
    vt_f = np.asarray(values, dtype=np.float32).transpose(0, 2, 1)
    kt_blk = kt_f.reshape(B, D, S // KB, KB)
    vt_blk = vt_f.reshape(B, D, S // KB, KB)

    def pack_w(W):
        # [DK, D] -> [128, DC*DK] with w[p, dc*DK+k] = W[k, dc*128+p]
        Wt = np.asarray(W, dtype=np.float32).T.reshape(DC, 128, DK)
        return np.ascontiguousarray(Wt.transpose(1, 0, 2).reshape(128, DC * DK)
                                    ).astype(bf16)

    wq, wk, wv = pack_w(Wq), pack_w(Wk), pack_w(Wv)

    in_maps = []
    for core in range(NCORES):
        b, h = divmod(core, 2)
        kth = np.ascontiguousarray(
            kt_blk[b, :, h::2, :].reshape(D, NKB * KB)).astype(bf16)
        vth = np.ascontiguousarray(
            vt_blk[b, :, h::2, :].reshape(D, NKB * KB)).astype(bf16)
        m = {"qt": qt[b], "kt": kth, "vt": vth,
             "wq": wq, "wk": wk, "wv": wv,
             "maskq": _mask_block(h).astype(bf16)}
        in_maps.append(m)

    global _last_in_maps
    _last_in_maps = in_maps
    nc = _get_prog("causal")
    res = run_bass_kernel_spmd(nc, in_maps, list(range(NCORES)))

    out = np.empty((B, S, DK), dtype=np.float32)
    ov = out.reshape(B, NCH, CH, DK)
    for b in range(B):
        r0 = np.asarray(res.results[2 * b]["out"], dtype=np.float32)
        r1 = np.asarray(res.results[2 * b + 1]["out"], dtype=np.float32)
        num = r0[:, :DK, :] + r1[:, :DK, :]  # [NCH, DK, CH]
        den = r0[:, DK:DK + 1, :] + r1[:, DK:DK + 1, :]  # [NCH, 1, CH]
        ov[b] = (num / den).transpose(0, 2, 1)
    return out


if __name__ == "__main__":
    rng = np.random.default_rng(0)
    q = rng.standard_normal((B, S, D), dtype=np.float32)
    k = rng.standard_normal((B, S, D), dtype=np.float32)
    v = rng.standard_normal((B, S, D), dtype=np.float32)
    sc = 1.0 / np.sqrt(D)
    wq = rng.uniform(-sc, sc, (DK, D)).astype(np.float32)
    wk = rng.uniform(-sc, sc, (DK, D)).astype(np.float32)
    wv = rng.uniform(-sc, sc, (DK, D)).astype(np.float32)
    msk = np.tril(np.ones((S, S), dtype=np.int32))
    out = kernel(queries=q, keys=k, values=v, Wq=wq, Wk=wk, Wv=wv, mask=msk)
    print("out", out.shape, out.dtype, float(np.abs(out).mean()))
